# revision 8
# baseline (speedup 1.0000x reference)
"""Multi-head attention (B=2, S=2048, E=1024, H=16, D=64) on 8 TRN2 NeuronCores.

Sharding: data parallel over batch (2) x head-group parallel (4 groups of 4
heads). Each core computes Q/K/V projections for its 4 heads, full-sequence
attention for those heads, and a partial output projection (its heads' rows of
Wo). The host sums the 4 partial outputs per batch and adds the bias.

On-device layout (all matmul operands bf16, accumulation fp32):
  - x is fed pre-transposed per batch: xT [E, S].
  - Q^T, K^T are computed head-transposed [dh, S] so that attention logits are
    produced key-partitioned: logitsT[k, q] = sum_d K^T[d,k] Q^T[d,q]. Softmax
    (no max subtraction -- logits are provably tiny) is exp on ScalarE plus a
    partition-reduction done with an all-ones matmul; attn@V then consumes the
    exp tiles directly as the moving operand with V [s, dh] as stationary.
  - The whole kernel runs the PE array in 64x64 tiling mode (tiles T0/T2/T8/
    T10) so that the d=64-contraction logits matmuls and everything else share
    one array mode (no mode-switch drains). Contractions >64 are split into
    row-group halves accumulated in two PSUM banks (alpha/beta) and combined
    on VectorE.
"""

import numpy as np
import ml_dtypes

import concourse.bass as bass
import concourse.mybir as mybir
import concourse.tile as tile
from concourse import bacc
from concourse import bass_utils
from contextlib import ExitStack

P = 128
B, S, E = 2, 2048, 1024
H, D = 16, 64
NCORES = 8
GROUPS = NCORES // B          # 4 head-groups per batch
HPG = H // GROUPS             # 4 heads per core
DHG = HPG * D                 # 256 head dims per core
NHP = HPG // 2                # 2 head-pairs per core
EC = E // P                   # 8 e-chunks of 128
KC = S // P                   # 16 key chunks of 128
QCW = 512                     # q-chunk width
NQC = S // QCW                # 4 q chunks
SCALE = float(D) ** -0.5

BF16 = mybir.dt.bfloat16
F32 = mybir.dt.float32
EXP = mybir.ActivationFunctionType.Exp

_NC = None


def _emit(tc):
    nc = tc.nc
    xT = nc.dram_tensor("xT", [E, S], BF16, kind="ExternalInput").ap()
    wqT = nc.dram_tensor("wqT", [E, DHG], BF16, kind="ExternalInput").ap()
    wkT = nc.dram_tensor("wkT", [E, DHG], BF16, kind="ExternalInput").ap()
    wvT = nc.dram_tensor("wvT", [E, DHG], BF16, kind="ExternalInput").ap()
    woT = nc.dram_tensor("woT", [DHG, E], BF16, kind="ExternalInput").ap()
    outT = nc.dram_tensor("outT", [E, S], F32, kind="ExternalOutput").ap()

    mm = nc.tensor.matmul

    with ExitStack() as ctx:
        consts = ctx.enter_context(tc.tile_pool(name="consts", bufs=1))
        xp = ctx.enter_context(tc.tile_pool(name="xp", bufs=1))
        qkvp = ctx.enter_context(tc.tile_pool(name="qkvp", bufs=1))
        psum = ctx.enter_context(tc.tile_pool(name="psum", bufs=3, space="PSUM"))
        psum_o = ctx.enter_context(tc.tile_pool(name="psum_o", bufs=1, space="PSUM"))
        expp = ctx.enter_context(tc.tile_pool(name="expp", bufs=2))
        s1p = ctx.enter_context(tc.tile_pool(name="s1p", bufs=1))
        smallp = ctx.enter_context(tc.tile_pool(name="smallp", bufs=2))
        otnp = ctx.enter_context(tc.tile_pool(name="otnp", bufs=4))
        outfp = ctx.enter_context(tc.tile_pool(name="outfp", bufs=3))

        # ---- load weights / x ----
        wq_sb = consts.tile([P, EC, DHG], BF16)
        wk_sb = consts.tile([P, EC, DHG], BF16)
        wv_sb = consts.tile([P, EC, DHG], BF16)
        wo_sb = consts.tile([P, NHP, E], BF16)
        ones = consts.tile([P, 64], BF16)
        nc.vector.memset(ones, 1.0)
        nc.sync.dma_start(wq_sb, wqT.rearrange("(c p) d -> p c d", p=P))
        nc.sync.dma_start(wk_sb, wkT.rearrange("(c p) d -> p c d", p=P))
        nc.sync.dma_start(wv_sb, wvT.rearrange("(c p) d -> p c d", p=P))
        nc.sync.dma_start(wo_sb, woT.rearrange("(h p) e -> p h e", p=P))

        x_sb = xp.tile([P, EC, S], BF16)
        for ec in range(EC):
            nc.sync.dma_start(x_sb[:, ec, :], xT[ec * P:(ec + 1) * P, :])

        # ---- phase 1: projections (plain 128x128 array mode) ----
        qt_sb = qkvp.tile([P, NHP, S], BF16)   # [dh-pair, hp, s]
        kt_sb = qkvp.tile([P, NHP, S], BF16)
        v_sb = qkvp.tile([P, KC, DHG], BF16)   # [s%128, s-chunk, dh]

        # Q^T / K^T: out[dh-pair, s]; two 512-wide s-chunks share one psum tile
        for hp in range(NHP):
            for w_sb, dst in ((wq_sb, qt_sb), (wk_sb, kt_sb)):
                for sc2 in range(NQC // 2):
                    ps = psum.tile([P, 2, QCW], F32, tag="ps")
                    for i in range(2):
                        ssl = slice((2 * sc2 + i) * QCW, (2 * sc2 + i + 1) * QCW)
                        for ec in range(EC):
                            mm(ps[:, i, :],
                               lhsT=w_sb[:, ec, hp * P:(hp + 1) * P],
                               rhs=x_sb[:, ec, ssl],
                               start=(ec == 0), stop=(ec == EC - 1),
                               skip_group_check=True)
                    dsl = dst[:, hp, 2 * sc2 * QCW:(2 * sc2 + 2) * QCW]
                    nc.vector.tensor_copy(
                        dsl.rearrange("p (c q) -> p c q", c=2), ps)

        # V: out[s-chunk, dh]; two 128-row s-chunks share one psum tile
        for sc2 in range(KC // 2):
            ps = psum.tile([P, 2, QCW], F32, tag="ps")
            for i in range(2):
                sc = 2 * sc2 + i
                for ec in range(EC):
                    mm(ps[:, i, 0:DHG],
                       lhsT=x_sb[:, ec, sc * P:(sc + 1) * P],
                       rhs=wv_sb[:, ec, :],
                       start=(ec == 0), stop=(ec == EC - 1),
                       skip_group_check=True)
            nc.vector.tensor_copy(v_sb[:, 2 * sc2:2 * sc2 + 2, :], ps[:, :, 0:DHG])

        # ---- phase 2: attention (64x64 array mode) + output projection ----
        otn_tiles = {}
        for qc in range(NQC):
            qsl = slice(qc * QCW, (qc + 1) * QCW)
            for hp in range(NHP):
                exp_t = expp.tile([P, 2, KC, QCW], BF16)
                psO = psum_o.tile([P, 2, QCW], F32)
                psL_tiles = [None] * KC
                for kcs in range(KC + 1):
                    if kcs < KC:
                        c = kcs
                        psL = psum.tile([P, 2, QCW], F32, tag="ps", name="psL")
                        psL_tiles[c] = psL
                        for h in range(2):      # head of the pair
                            hg = slice(h * 64, (h + 1) * 64)
                            for p2 in range(2):  # key sub-chunk of 64
                                mm(psL[p2 * 64:(p2 + 1) * 64, h, :],
                                   lhsT=kt_sb[hg, hp, c * P + p2 * 64:c * P + (p2 + 1) * 64],
                                   rhs=qt_sb[hg, hp, qsl],
                                   start=True, stop=True,
                                   tile_position=(h * 64, p2 * 64), skip_group_check=True)
                        nc.scalar.activation(exp_t[:, :, c, :], psL, EXP, scale=SCALE)
                    if kcs >= 1:
                        c = kcs - 1
                        for h in range(2):
                            col = hp * P + h * 64
                            for g in range(2):  # key sub-chunk = row group
                                rg = slice(g * 64, (g + 1) * 64)
                                mm(psO[h * 64:(h + 1) * 64, g, :],
                                   lhsT=v_sb[rg, c, col:col + 64],
                                   rhs=exp_t[rg, h, c, :],
                                   start=(c == 0), stop=(c == KC - 1),
                                   tile_position=(g * 64, h * 64), skip_group_check=True)

                # softmax denominators: one bf16 tree level (16->8), then
                # all-ones matmuls reduce+broadcast across key partitions
                s1 = s1p.tile([P, 2, 8, QCW], BF16)
                nc.vector.tensor_add(s1, exp_t[:, :, 0:8, :], exp_t[:, :, 8:16, :])
                psS = psum.tile([P, 2, QCW], F32, tag="ps", name="psS")
                for j in range(8):
                    for h in range(2):
                        for g in range(2):
                            rg = slice(g * 64, (g + 1) * 64)
                            mm(psS[h * 64:(h + 1) * 64, g, :],
                               lhsT=ones[rg, :],
                               rhs=s1[rg, h, j, :],
                               start=(j == 0), stop=(j == 7),
                               tile_position=(g * 64, h * 64), skip_group_check=True)
                # rec = 1 / (psS_alpha + psS_beta)   (single-PSUM-operand ops)
                st = smallp.tile([P, QCW], F32, tag="st")
                nc.vector.tensor_copy(st, psS[:, 0, :])
                ssum = smallp.tile([P, QCW], F32, tag="ssum")
                nc.vector.tensor_add(ssum, psS[:, 1, :], st)
                rec = smallp.tile([P, QCW], F32, tag="rec")
                nc.vector.reciprocal(rec, ssum)
                # otn = (psO_alpha + psO_beta) * rec, cast to bf16
                ot = smallp.tile([P, QCW], F32, tag="ot")
                nc.vector.tensor_copy(ot, psO[:, 0, :])
                otf = smallp.tile([P, QCW], F32, tag="otf")
                nc.vector.tensor_add(otf, psO[:, 1, :], ot)
                otn = otnp.tile([P, QCW], BF16)
                nc.vector.tensor_mul(otn, otf, rec)
                otn_tiles[(hp, qc)] = otn

            # output projection for this q-chunk (128x128 array mode);
            # two 128-row eo-chunks share one psum tile
            for m2 in range(EC // 2):
                psP = psum.tile([P, 2, QCW], F32, tag="ps", name="psP")
                for i in range(2):
                    m = 2 * m2 + i
                    for hp2 in range(NHP):
                        mm(psP[:, i, :],
                           lhsT=wo_sb[:, hp2, m * P:(m + 1) * P],
                           rhs=otn_tiles[(hp2, qc)],
                           start=(hp2 == 0), stop=(hp2 == NHP - 1),
                           skip_group_check=True)
                outf = outfp.tile([P, 2, QCW], F32)
                nc.vector.tensor_copy(outf, psP)
                nc.sync.dma_start(
                    outT[2 * m2 * P:(2 * m2 + 2) * P, qsl].rearrange(
                        "(c p) q -> p c q", p=P),
                    outf)


def _build():
    nc = bacc.Bacc("TRN2", debug=False, target_bir_lowering=False)
    with tile.TileContext(nc) as tc:
        _emit(tc)
    nc.compile()
    return nc


def _get_nc():
    global _NC
    if _NC is None:
        _NC = _build()
    return _NC


def make_in_maps(x, Wq, Wk, Wv, Wo):
    bf = ml_dtypes.bfloat16
    x = np.asarray(x, np.float32)
    xTb = [np.ascontiguousarray(x[b].T).astype(bf) for b in range(B)]
    WqT = np.ascontiguousarray(np.asarray(Wq, np.float32).T).astype(bf)
    WkT = np.ascontiguousarray(np.asarray(Wk, np.float32).T).astype(bf)
    WvT = np.ascontiguousarray(np.asarray(Wv, np.float32).T).astype(bf)
    WoT = np.ascontiguousarray(np.asarray(Wo, np.float32).T).astype(bf)

    in_maps = []
    for c in range(NCORES):
        b, hg = divmod(c, GROUPS)
        sl = slice(hg * DHG, (hg + 1) * DHG)
        in_maps.append({
            "xT": xTb[b],
            "wqT": np.ascontiguousarray(WqT[:, sl]),
            "wkT": np.ascontiguousarray(WkT[:, sl]),
            "wvT": np.ascontiguousarray(WvT[:, sl]),
            "woT": np.ascontiguousarray(WoT[sl, :]),
        })
    return in_maps


def run(in_maps, **kwargs):
    nc = _get_nc()
    return bass_utils.run_bass_kernel_spmd(
        nc, in_maps, core_ids=list(range(NCORES)), **kwargs)


def assemble(outs, bo):
    bo = np.asarray(bo, np.float32)
    out = np.empty((B, S, E), np.float32)
    for b in range(B):
        acc = outs[b * GROUPS]["outT"].copy()
        for hg in range(1, GROUPS):
            acc += outs[b * GROUPS + hg]["outT"]
        out[b] = acc.T + bo
    return out


def kernel(x, Wq, Wk, Wv, Wo, bo):
    in_maps = make_in_maps(x, Wq, Wk, Wv, Wo)
    res = run(in_maps)
    return assemble(res.results, bo)


# revision 10
# speedup vs baseline: 1.0970x; 1.0970x over previous
"""Multi-head attention (B=2, S=2048, E=1024, H=16, D=64) on 8 TRN2 NeuronCores.

Sharding: data parallel over batch (2) x head-group parallel (4 groups of 4
heads). Each core computes Q/K/V projections for its 4 heads, full-sequence
attention for those heads, and a partial output projection (its heads' rows of
Wo). The host sums the 4 partial outputs per batch and adds the bias.

On-device layout (all matmul operands bf16, accumulation fp32):
  - x is fed pre-transposed per batch: xT [E, S].
  - Q^T, K^T are computed head-transposed [dh, S] so that attention logits are
    produced key-partitioned: logitsT[k, q] = sum_d K^T[d,k] Q^T[d,q]. Softmax
    (no max subtraction -- logits are provably tiny) is exp on ScalarE plus a
    partition-reduction done with an all-ones matmul; attn@V then consumes the
    exp tiles directly as the moving operand with V [s, dh] as stationary.
  - The whole kernel runs the PE array in 64x64 tiling mode (tiles T0/T2/T8/
    T10) so that the d=64-contraction logits matmuls and everything else share
    one array mode (no mode-switch drains). Contractions >64 are split into
    row-group halves accumulated in two PSUM banks (alpha/beta) and combined
    on VectorE.
"""

import numpy as np
import ml_dtypes

import concourse.bass as bass
import concourse.mybir as mybir
import concourse.tile as tile
from concourse import bacc
from concourse import bass_utils
from contextlib import ExitStack

P = 128
B, S, E = 2, 2048, 1024
H, D = 16, 64
NCORES = 8
GROUPS = NCORES // B          # 4 head-groups per batch
HPG = H // GROUPS             # 4 heads per core
DHG = HPG * D                 # 256 head dims per core
NHP = HPG // 2                # 2 head-pairs per core
EC = E // P                   # 8 e-chunks of 128
KC = S // P                   # 16 key chunks of 128
QCW = 512                     # q-chunk width
NQC = S // QCW                # 4 q chunks
SCALE = float(D) ** -0.5

BF16 = mybir.dt.bfloat16
F32 = mybir.dt.float32
EXP = mybir.ActivationFunctionType.Exp

_NC = None


def _emit(tc):
    nc = tc.nc
    xT = nc.dram_tensor("xT", [E, S], BF16, kind="ExternalInput").ap()
    wqT = nc.dram_tensor("wqT", [E, DHG], BF16, kind="ExternalInput").ap()
    wkT = nc.dram_tensor("wkT", [E, DHG], BF16, kind="ExternalInput").ap()
    wvT = nc.dram_tensor("wvT", [E, DHG], BF16, kind="ExternalInput").ap()
    woT = nc.dram_tensor("woT", [DHG, E], BF16, kind="ExternalInput").ap()
    outT = nc.dram_tensor("outT", [E, S], F32, kind="ExternalOutput").ap()

    mm = nc.tensor.matmul

    with ExitStack() as ctx:
        consts = ctx.enter_context(tc.tile_pool(name="consts", bufs=1))
        xp = ctx.enter_context(tc.tile_pool(name="xp", bufs=1))
        qkvp = ctx.enter_context(tc.tile_pool(name="qkvp", bufs=1))
        psum = ctx.enter_context(tc.tile_pool(name="psum", bufs=2, space="PSUM"))
        psum_x = ctx.enter_context(tc.tile_pool(name="psum_x", bufs=1, space="PSUM"))
        psum_o = ctx.enter_context(tc.tile_pool(name="psum_o", bufs=1, space="PSUM"))
        expp = ctx.enter_context(tc.tile_pool(name="expp", bufs=2))
        s1p = ctx.enter_context(tc.tile_pool(name="s1p", bufs=1))
        smallp = ctx.enter_context(tc.tile_pool(name="smallp", bufs=2))
        otnp = ctx.enter_context(tc.tile_pool(name="otnp", bufs=4))
        outfp = ctx.enter_context(tc.tile_pool(name="outfp", bufs=3))

        # ---- load weights / x ----
        wq_sb = consts.tile([P, EC, DHG], BF16)
        wk_sb = consts.tile([P, EC, DHG], BF16)
        wv_sb = consts.tile([P, EC, DHG], BF16)
        wo_sb = consts.tile([P, NHP, E], BF16)
        ones = consts.tile([P, 64], BF16)
        nc.vector.memset(ones, 1.0)
        nc.sync.dma_start(wq_sb, wqT.rearrange("(c p) d -> p c d", p=P))
        nc.scalar.dma_start(wk_sb, wkT.rearrange("(c p) d -> p c d", p=P))

        x_sb = xp.tile([P, EC, S], BF16)
        dma_eng = [nc.sync, nc.gpsimd, nc.scalar]
        for ec in range(EC):
            dma_eng[ec % 3].dma_start(x_sb[:, ec, :], xT[ec * P:(ec + 1) * P, :])
        nc.gpsimd.dma_start(wv_sb, wvT.rearrange("(c p) d -> p c d", p=P))
        nc.gpsimd.dma_start(wo_sb, woT.rearrange("(h p) e -> p h e", p=P))

        # ---- phase 1: projections (plain 128x128 array mode) ----
        qt_sb = qkvp.tile([P, NHP, S], BF16)   # [dh-pair, hp, s]
        kt_sb = qkvp.tile([P, NHP, S], BF16)
        v_sb = qkvp.tile([P, KC, DHG], BF16)   # [s%128, s-chunk, dh]

        # Q^T / K^T: out[dh-pair, s]; two 512-wide s-chunks share one psum tile
        for hp in range(NHP):
            for w_sb, dst in ((wq_sb, qt_sb), (wk_sb, kt_sb)):
                for sc2 in range(NQC // 2):
                    ps = psum.tile([P, 2, QCW], F32, tag="ps")
                    for i in range(2):
                        ssl = slice((2 * sc2 + i) * QCW, (2 * sc2 + i + 1) * QCW)
                        for ec in range(EC):
                            mm(ps[:, i, :],
                               lhsT=w_sb[:, ec, hp * P:(hp + 1) * P],
                               rhs=x_sb[:, ec, ssl],
                               start=(ec == 0), stop=(ec == EC - 1),
                               skip_group_check=True)
                    dsl = dst[:, hp, 2 * sc2 * QCW:(2 * sc2 + 2) * QCW]
                    nc.vector.tensor_copy(
                        dsl.rearrange("p (c q) -> p c q", c=2), ps)

        # V: out[s-chunk, dh]; two 128-row s-chunks share one psum tile
        for sc2 in range(KC // 2):
            ps = psum.tile([P, 2, QCW], F32, tag="ps")
            for i in range(2):
                sc = 2 * sc2 + i
                for ec in range(EC):
                    mm(ps[:, i, 0:DHG],
                       lhsT=x_sb[:, ec, sc * P:(sc + 1) * P],
                       rhs=wv_sb[:, ec, :],
                       start=(ec == 0), stop=(ec == EC - 1),
                       skip_group_check=True)
            nc.vector.tensor_copy(v_sb[:, 2 * sc2:2 * sc2 + 2, :], ps[:, :, 0:DHG])

        # ---- phase 2: attention (64x64 array mode) + output projection ----
        otn_tiles = {}
        for qc in range(NQC):
            qsl = slice(qc * QCW, (qc + 1) * QCW)
            for hp in range(NHP):
                exp_t = expp.tile([P, 2, KC, QCW], BF16)
                psO = psum_o.tile([P, 2, QCW], F32)
                psL_tiles = [None] * KC
                for kcs in range(KC + 1):
                    if kcs < KC:
                        c = kcs
                        psL = psum.tile([P, 2, QCW], F32, tag="ps", name="psL")
                        psL_tiles[c] = psL
                        for h in range(2):      # head of the pair
                            hg = slice(h * 64, (h + 1) * 64)
                            for p2 in range(2):  # key sub-chunk of 64
                                mm(psL[p2 * 64:(p2 + 1) * 64, h, :],
                                   lhsT=kt_sb[hg, hp, c * P + p2 * 64:c * P + (p2 + 1) * 64],
                                   rhs=qt_sb[hg, hp, qsl],
                                   start=True, stop=True,
                                   tile_position=(h * 64, p2 * 64), skip_group_check=True)
                        nc.scalar.activation(exp_t[:, :, c, :], psL, EXP, scale=SCALE)
                    if kcs >= 1:
                        c = kcs - 1
                        for h in range(2):
                            col = hp * P + h * 64
                            for g in range(2):  # key sub-chunk = row group
                                rg = slice(g * 64, (g + 1) * 64)
                                mm(psO[h * 64:(h + 1) * 64, g, :],
                                   lhsT=v_sb[rg, c, col:col + 64],
                                   rhs=exp_t[rg, h, c, :],
                                   start=(c == 0), stop=(c == KC - 1),
                                   tile_position=(g * 64, h * 64), skip_group_check=True)

                # softmax denominators: one bf16 tree level (16->8), then
                # all-ones matmuls reduce+broadcast across key partitions
                s1 = s1p.tile([P, 2, 8, QCW], BF16)
                nc.vector.tensor_add(s1, exp_t[:, :, 0:8, :], exp_t[:, :, 8:16, :])
                psS = psum_x.tile([P, 2, QCW], F32, tag="psx", name="psS")
                for j in range(8):
                    for h in range(2):
                        for g in range(2):
                            rg = slice(g * 64, (g + 1) * 64)
                            mm(psS[h * 64:(h + 1) * 64, g, :],
                               lhsT=ones[rg, :],
                               rhs=s1[rg, h, j, :],
                               start=(j == 0), stop=(j == 7),
                               tile_position=(g * 64, h * 64), skip_group_check=True)
                # rec = 1 / (psS_alpha + psS_beta)   (single-PSUM-operand ops)
                st = smallp.tile([P, QCW], F32, tag="st")
                nc.vector.tensor_copy(st, psS[:, 0, :])
                ssum = smallp.tile([P, QCW], F32, tag="ssum")
                nc.vector.tensor_add(ssum, psS[:, 1, :], st)
                rec = smallp.tile([P, QCW], F32, tag="rec")
                rscr = smallp.tile([P, QCW], F32, tag="rscr")
                nc.vector.reciprocal_approx_accurate(rec, ssum, rscr)
                # otn = (psO_alpha + psO_beta) * rec, cast to bf16
                ot = smallp.tile([P, QCW], F32, tag="ot")
                nc.vector.tensor_copy(ot, psO[:, 0, :])
                otf = smallp.tile([P, QCW], F32, tag="otf")
                nc.vector.tensor_add(otf, psO[:, 1, :], ot)
                otn = otnp.tile([P, QCW], BF16)
                nc.vector.tensor_mul(otn, otf, rec)
                otn_tiles[(hp, qc)] = otn

            # output projection for this q-chunk (128x128 array mode);
            # two 128-row eo-chunks share one psum tile
            for m2 in range(EC // 2):
                psP = psum_x.tile([P, 2, QCW], F32, tag="psx", name="psP")
                for i in range(2):
                    m = 2 * m2 + i
                    for hp2 in range(NHP):
                        mm(psP[:, i, :],
                           lhsT=wo_sb[:, hp2, m * P:(m + 1) * P],
                           rhs=otn_tiles[(hp2, qc)],
                           start=(hp2 == 0), stop=(hp2 == NHP - 1),
                           skip_group_check=True)
                outf = outfp.tile([P, 2, QCW], F32)
                nc.vector.tensor_copy(outf, psP)
                (nc.sync if m2 % 2 == 0 else nc.gpsimd).dma_start(
                    outT[2 * m2 * P:(2 * m2 + 2) * P, qsl].rearrange(
                        "(c p) q -> p c q", p=P),
                    outf)


def _build():
    nc = bacc.Bacc("TRN2", debug=False, target_bir_lowering=False)
    with tile.TileContext(nc) as tc:
        _emit(tc)
    nc.compile()
    return nc


def _get_nc():
    global _NC
    if _NC is None:
        _NC = _build()
    return _NC


def make_in_maps(x, Wq, Wk, Wv, Wo):
    bf = ml_dtypes.bfloat16
    x = np.asarray(x, np.float32)
    xTb = [np.ascontiguousarray(x[b].T).astype(bf) for b in range(B)]
    WqT = np.ascontiguousarray(np.asarray(Wq, np.float32).T).astype(bf)
    WkT = np.ascontiguousarray(np.asarray(Wk, np.float32).T).astype(bf)
    WvT = np.ascontiguousarray(np.asarray(Wv, np.float32).T).astype(bf)
    WoT = np.ascontiguousarray(np.asarray(Wo, np.float32).T).astype(bf)

    in_maps = []
    for c in range(NCORES):
        b, hg = divmod(c, GROUPS)
        sl = slice(hg * DHG, (hg + 1) * DHG)
        in_maps.append({
            "xT": xTb[b],
            "wqT": np.ascontiguousarray(WqT[:, sl]),
            "wkT": np.ascontiguousarray(WkT[:, sl]),
            "wvT": np.ascontiguousarray(WvT[:, sl]),
            "woT": np.ascontiguousarray(WoT[sl, :]),
        })
    return in_maps


def run(in_maps, **kwargs):
    nc = _get_nc()
    return bass_utils.run_bass_kernel_spmd(
        nc, in_maps, core_ids=list(range(NCORES)), **kwargs)


def assemble(outs, bo):
    bo = np.asarray(bo, np.float32)
    out = np.empty((B, S, E), np.float32)
    for b in range(B):
        acc = outs[b * GROUPS]["outT"].copy()
        for hg in range(1, GROUPS):
            acc += outs[b * GROUPS + hg]["outT"]
        out[b] = acc.T + bo
    return out


def kernel(x, Wq, Wk, Wv, Wo, bo):
    in_maps = make_in_maps(x, Wq, Wk, Wv, Wo)
    res = run(in_maps)
    return assemble(res.results, bo)


# revision 11
# speedup vs baseline: 1.1168x; 1.0180x over previous
"""Multi-head attention (B=2, S=2048, E=1024, H=16, D=64) on 8 TRN2 NeuronCores.

Sharding: data parallel over batch (2) x head-group parallel (4 groups of 4
heads). Each core computes Q/K/V projections for its 4 heads, full-sequence
attention for those heads, and a partial output projection (its heads' rows of
Wo). The host sums the 4 partial outputs per batch and adds the bias.

On-device layout (all matmul operands bf16, accumulation fp32):
  - x is fed pre-transposed per batch: xT [E, S].
  - Q^T, K^T are computed head-transposed [dh, S] so that attention logits are
    produced key-partitioned: logitsT[k, q] = sum_d K^T[d,k] Q^T[d,q]. Softmax
    (no max subtraction -- logits are provably tiny) is exp on ScalarE plus a
    partition-reduction done with an all-ones matmul; attn@V then consumes the
    exp tiles directly as the moving operand with V [s, dh] as stationary.
  - The whole kernel runs the PE array in 64x64 tiling mode (tiles T0/T2/T8/
    T10) so that the d=64-contraction logits matmuls and everything else share
    one array mode (no mode-switch drains). Contractions >64 are split into
    row-group halves accumulated in two PSUM banks (alpha/beta) and combined
    on VectorE.
"""

import numpy as np
import ml_dtypes

import concourse.bass as bass
import concourse.mybir as mybir
import concourse.tile as tile
from concourse import bacc
from concourse import bass_utils
from contextlib import ExitStack

P = 128
B, S, E = 2, 2048, 1024
H, D = 16, 64
NCORES = 8
GROUPS = NCORES // B          # 4 head-groups per batch
HPG = H // GROUPS             # 4 heads per core
DHG = HPG * D                 # 256 head dims per core
NHP = HPG // 2                # 2 head-pairs per core
EC = E // P                   # 8 e-chunks of 128
KC = S // P                   # 16 key chunks of 128
QCW = 512                     # q-chunk width
NQC = S // QCW                # 4 q chunks
SCALE = float(D) ** -0.5

BF16 = mybir.dt.bfloat16
F32 = mybir.dt.float32
EXP = mybir.ActivationFunctionType.Exp

_NC = None


def _emit(tc):
    nc = tc.nc
    xT = nc.dram_tensor("xT", [E, S], BF16, kind="ExternalInput").ap()
    wqT = nc.dram_tensor("wqT", [E, DHG], BF16, kind="ExternalInput").ap()
    wkT = nc.dram_tensor("wkT", [E, DHG], BF16, kind="ExternalInput").ap()
    wvT = nc.dram_tensor("wvT", [E, DHG], BF16, kind="ExternalInput").ap()
    woT = nc.dram_tensor("woT", [DHG, E], BF16, kind="ExternalInput").ap()
    outT = nc.dram_tensor("outT", [E, S], F32, kind="ExternalOutput").ap()

    mm = nc.tensor.matmul

    with ExitStack() as ctx:
        consts = ctx.enter_context(tc.tile_pool(name="consts", bufs=1))
        xp = ctx.enter_context(tc.tile_pool(name="xp", bufs=1))
        qkvp = ctx.enter_context(tc.tile_pool(name="qkvp", bufs=1))
        psum = ctx.enter_context(tc.tile_pool(name="psum", bufs=3, space="PSUM"))
        psum_o = ctx.enter_context(tc.tile_pool(name="psum_o", bufs=1, space="PSUM"))
        expp = ctx.enter_context(tc.tile_pool(name="expp", bufs=2))
        s1p = ctx.enter_context(tc.tile_pool(name="s1p", bufs=1))
        smallp = ctx.enter_context(tc.tile_pool(name="smallp", bufs=2))
        otnp = ctx.enter_context(tc.tile_pool(name="otnp", bufs=6))
        outfp = ctx.enter_context(tc.tile_pool(name="outfp", bufs=3))

        # ---- input loads: wv + x first (V runs first), spread across queues
        wq_sb = consts.tile([P, EC, DHG], BF16)
        wk_sb = consts.tile([P, EC, DHG], BF16)
        wv_sb = consts.tile([P, EC, DHG], BF16)
        wo_sb = consts.tile([P, NHP, E], BF16)
        ones = consts.tile([P, 64], BF16)
        nc.vector.memset(ones, 1.0)
        nc.scalar.dma_start(wv_sb, wvT.rearrange("(c p) d -> p c d", p=P))
        x_sb = xp.tile([P, EC, S], BF16)
        dma_eng = [nc.gpsimd, nc.sync, nc.scalar]
        for ec in range(EC):
            dma_eng[ec % 3].dma_start(x_sb[:, ec, :], xT[ec * P:(ec + 1) * P, :])
        nc.sync.dma_start(wq_sb, wqT.rearrange("(c p) d -> p c d", p=P))
        nc.gpsimd.dma_start(wk_sb, wkT.rearrange("(c p) d -> p c d", p=P))
        nc.gpsimd.dma_start(wo_sb, woT.rearrange("(h p) e -> p h e", p=P))

        qt_sb = qkvp.tile([P, NHP, S], BF16)   # [dh-pair, hp, s]
        kt_sb = qkvp.tile([P, NHP, S], BF16)
        v_sb = qkvp.tile([P, KC, DHG], BF16)   # [s%128, s-chunk, dh]

        # deferred-work queue: (approx_pe_ns, closure) drained into kc slots
        pend = []

        def drain(budget):
            spent = 0
            while pend and spent < budget:
                cost, fn = pend.pop(0)
                fn()
                spent += cost

        # ---- phase 1 emitters (plain 128x128 array mode) ----
        def emit_v(sc2):
            ps = psum.tile([P, 2, QCW], F32, tag="ps", name="psV")
            for i in range(2):
                sc = 2 * sc2 + i
                for ec in range(EC):
                    mm(ps[:, i, 0:DHG],
                       lhsT=x_sb[:, ec, sc * P:(sc + 1) * P],
                       rhs=wv_sb[:, ec, :],
                       start=(ec == 0), stop=(ec == EC - 1),
                       skip_group_check=True)
            nc.vector.tensor_copy(v_sb[:, 2 * sc2:2 * sc2 + 2, :], ps[:, :, 0:DHG])

        def emit_qk(hp, w_sb, dst, sc2):
            ps = psum.tile([P, 2, QCW], F32, tag="ps", name="psQK")
            for i in range(2):
                ssl = slice((2 * sc2 + i) * QCW, (2 * sc2 + i + 1) * QCW)
                for ec in range(EC):
                    mm(ps[:, i, :],
                       lhsT=w_sb[:, ec, hp * P:(hp + 1) * P],
                       rhs=x_sb[:, ec, ssl],
                       start=(ec == 0), stop=(ec == EC - 1),
                       skip_group_check=True)
            dsl = dst[:, hp, 2 * sc2 * QCW:(2 * sc2 + 2) * QCW]
            nc.vector.tensor_copy(dsl.rearrange("p (c q) -> p c q", c=2), ps)

        # serial prologue: V + Q/K for head-pair 0
        for sc2 in range(KC // 2):
            emit_v(sc2)
        for w_sb, dst in ((wq_sb, qt_sb), (wk_sb, kt_sb)):
            for sc2 in range(NQC // 2):
                emit_qk(0, w_sb, dst, sc2)
        # Q/K for head-pair 1: deferred into head-pair 0's attention slots
        for w_sb, dst in ((wq_sb, qt_sb), (wk_sb, kt_sb)):
            for sc2 in range(NQC // 2):
                pend.append((3400, lambda w=w_sb, d=dst, s=sc2: emit_qk(1, w, d, s)))

        # ---- phase 2: attention (64x64 array mode), deferred tails ----
        otn_tiles = {}

        def make_tail(hp, qc, exp_t, psO):
            def evac():
                ot = smallp.tile([P, QCW], F32, tag="ot")
                nc.vector.tensor_copy(ot, psO[:, 0, :])
                otf = smallp.tile([P, QCW], F32, tag="otf")
                nc.vector.tensor_add(otf, psO[:, 1, :], ot)
                otn_tiles[(hp, qc)] = (otf, None)

            def tree():
                s1 = s1p.tile([P, 2, 8, QCW], BF16)
                nc.vector.tensor_add(s1, exp_t[:, :, 0:8, :], exp_t[:, :, 8:16, :])
                psS = psum.tile([P, 2, QCW], F32, tag="ps", name="psS")
                otn_tiles[(hp, qc, "s")] = (s1, psS)

            def ones_blk(j0):
                s1, psS = otn_tiles[(hp, qc, "s")]
                for j in range(j0, j0 + 4):
                    for h in range(2):
                        for g in range(2):
                            rg = slice(g * 64, (g + 1) * 64)
                            mm(psS[h * 64:(h + 1) * 64, g, :],
                               lhsT=ones[rg, :],
                               rhs=s1[rg, h, j, :],
                               start=(j == 0), stop=(j == 7),
                               tile_position=(g * 64, h * 64),
                               skip_group_check=True)

            def norm():
                _, psS = otn_tiles[(hp, qc, "s")]
                st = smallp.tile([P, QCW], F32, tag="st")
                nc.vector.tensor_copy(st, psS[:, 0, :])
                ssum = smallp.tile([P, QCW], F32, tag="ssum")
                nc.vector.tensor_add(ssum, psS[:, 1, :], st)
                rec = smallp.tile([P, QCW], F32, tag="rec")
                rscr = smallp.tile([P, QCW], F32, tag="rscr")
                nc.vector.reciprocal_approx_accurate(rec, ssum, rscr)
                otf, _ = otn_tiles[(hp, qc)]
                otn = otnp.tile([P, QCW], BF16)
                nc.vector.tensor_mul(otn, otf, rec)
                otn_tiles[(hp, qc)] = otn

            return [(200, evac), (150, tree),
                    (900, lambda: ones_blk(0)), (900, lambda: ones_blk(4)),
                    (350, norm)]

        def make_outproj(qc):
            qsl = slice(qc * QCW, (qc + 1) * QCW)

            def blk(m2):
                psP = psum.tile([P, 2, QCW], F32, tag="ps", name="psP")
                for i in range(2):
                    m = 2 * m2 + i
                    for hp2 in range(NHP):
                        mm(psP[:, i, :],
                           lhsT=wo_sb[:, hp2, m * P:(m + 1) * P],
                           rhs=otn_tiles[(hp2, qc)],
                           start=(hp2 == 0), stop=(hp2 == NHP - 1),
                           skip_group_check=True)
                outf = outfp.tile([P, 2, QCW], F32)
                nc.vector.tensor_copy(outf, psP)
                (nc.sync if m2 % 2 == 0 else nc.gpsimd).dma_start(
                    outT[2 * m2 * P:(2 * m2 + 2) * P, qsl].rearrange(
                        "(c p) q -> p c q", p=P),
                    outf)

            return [(1900, lambda m=m2: blk(m)) for m2 in range(EC // 2)]

        for hp in range(NHP):
            for qc in range(NQC):
                qsl = slice(qc * QCW, (qc + 1) * QCW)
                exp_t = expp.tile([P, 2, KC, QCW], BF16)
                psO = psum_o.tile([P, 2, QCW], F32)
                for kcs in range(KC + 1):
                    if kcs < KC:
                        c = kcs
                        psL = psum.tile([P, 2, QCW], F32, tag="ps", name="psL")
                        for h in range(2):      # head of the pair
                            hg = slice(h * 64, (h + 1) * 64)
                            for p2 in range(2):  # key sub-chunk of 64
                                mm(psL[p2 * 64:(p2 + 1) * 64, h, :],
                                   lhsT=kt_sb[hg, hp, c * P + p2 * 64:c * P + (p2 + 1) * 64],
                                   rhs=qt_sb[hg, hp, qsl],
                                   start=True, stop=True,
                                   tile_position=(h * 64, p2 * 64),
                                   skip_group_check=True)
                        nc.scalar.activation(exp_t[:, :, c, :], psL, EXP, scale=SCALE)
                    drain(700)
                    if kcs >= 1:
                        c = kcs - 1
                        for h in range(2):
                            col = hp * P + h * 64
                            for g in range(2):  # key sub-chunk = row group
                                rg = slice(g * 64, (g + 1) * 64)
                                mm(psO[h * 64:(h + 1) * 64, g, :],
                                   lhsT=v_sb[rg, c, col:col + 64],
                                   rhs=exp_t[rg, h, c, :],
                                   start=(c == 0), stop=(c == KC - 1),
                                   tile_position=(g * 64, h * 64),
                                   skip_group_check=True)
                pend.extend(make_tail(hp, qc, exp_t, psO))
                if hp == 1:
                    pend.extend(make_outproj(qc))
        drain(10**9)


def _build():
    nc = bacc.Bacc("TRN2", debug=False, target_bir_lowering=False)
    with tile.TileContext(nc) as tc:
        _emit(tc)
    nc.compile()
    return nc


def _get_nc():
    global _NC
    if _NC is None:
        _NC = _build()
    return _NC


def make_in_maps(x, Wq, Wk, Wv, Wo):
    bf = ml_dtypes.bfloat16
    x = np.asarray(x, np.float32)
    xTb = [np.ascontiguousarray(x[b].T).astype(bf) for b in range(B)]
    WqT = np.ascontiguousarray(np.asarray(Wq, np.float32).T).astype(bf)
    WkT = np.ascontiguousarray(np.asarray(Wk, np.float32).T).astype(bf)
    WvT = np.ascontiguousarray(np.asarray(Wv, np.float32).T).astype(bf)
    WoT = np.ascontiguousarray(np.asarray(Wo, np.float32).T).astype(bf)

    in_maps = []
    for c in range(NCORES):
        b, hg = divmod(c, GROUPS)
        sl = slice(hg * DHG, (hg + 1) * DHG)
        in_maps.append({
            "xT": xTb[b],
            "wqT": np.ascontiguousarray(WqT[:, sl]),
            "wkT": np.ascontiguousarray(WkT[:, sl]),
            "wvT": np.ascontiguousarray(WvT[:, sl]),
            "woT": np.ascontiguousarray(WoT[sl, :]),
        })
    return in_maps


def run(in_maps, **kwargs):
    nc = _get_nc()
    return bass_utils.run_bass_kernel_spmd(
        nc, in_maps, core_ids=list(range(NCORES)), **kwargs)


def assemble(outs, bo):
    bo = np.asarray(bo, np.float32)
    out = np.empty((B, S, E), np.float32)
    for b in range(B):
        acc = outs[b * GROUPS]["outT"].copy()
        for hg in range(1, GROUPS):
            acc += outs[b * GROUPS + hg]["outT"]
        out[b] = acc.T + bo
    return out


def kernel(x, Wq, Wk, Wv, Wo, bo):
    in_maps = make_in_maps(x, Wq, Wk, Wv, Wo)
    res = run(in_maps)
    return assemble(res.results, bo)


# revision 13
# speedup vs baseline: 1.3486x; 1.2076x over previous
"""Multi-head attention (B=2, S=2048, E=1024, H=16, D=64) on 8 TRN2 NeuronCores.

Sharding: data parallel over batch (2) x head-group parallel (4 groups of 4
heads). Each core computes Q/K/V projections for its 4 heads, full-sequence
attention for those heads, and a partial output projection (its heads' rows of
Wo). The host sums the 4 partial outputs per batch and adds the bias.

On-device layout (all matmul operands bf16, accumulation fp32):
  - x is fed pre-transposed per batch: xT [E, S].
  - Q^T, K^T are computed head-transposed [dh, S] so that attention logits are
    produced key-partitioned: logitsT[k, q] = sum_d K^T[d,k] Q^T[d,q]. Softmax
    (no max subtraction -- logits are provably tiny) is exp on ScalarE plus a
    partition-reduction done with an all-ones matmul; attn@V then consumes the
    exp tiles directly as the moving operand with V [s, dh] as stationary.
  - The whole kernel runs the PE array in 64x64 tiling mode (tiles T0/T2/T8/
    T10) so that the d=64-contraction logits matmuls and everything else share
    one array mode (no mode-switch drains). Contractions >64 are split into
    row-group halves accumulated in two PSUM banks (alpha/beta) and combined
    on VectorE.
"""

import numpy as np
import ml_dtypes

import concourse.bass as bass
import concourse.mybir as mybir
import concourse.tile as tile
from concourse import bacc
from concourse import bass_utils
from contextlib import ExitStack

P = 128
B, S, E = 2, 2048, 1024
H, D = 16, 64
NCORES = 8
GROUPS = NCORES // B          # 4 head-groups per batch
HPG = H // GROUPS             # 4 heads per core
DHG = HPG * D                 # 256 head dims per core
NHP = HPG // 2                # 2 head-pairs per core
EC = E // P                   # 8 e-chunks of 128
KC = S // P                   # 16 key chunks of 128
QCW = 512                     # q-chunk width
NQC = S // QCW                # 4 q chunks
SCALE = float(D) ** -0.5

BF16 = mybir.dt.bfloat16
F32 = mybir.dt.float32
EXP = mybir.ActivationFunctionType.Exp

_NC = None


def _emit(tc):
    nc = tc.nc
    xT = nc.dram_tensor("xT", [E, S], BF16, kind="ExternalInput").ap()
    wqT = nc.dram_tensor("wqT", [E, DHG], BF16, kind="ExternalInput").ap()
    wkT = nc.dram_tensor("wkT", [E, DHG], BF16, kind="ExternalInput").ap()
    wvT = nc.dram_tensor("wvT", [E, DHG], BF16, kind="ExternalInput").ap()
    woT = nc.dram_tensor("woT", [DHG, E], BF16, kind="ExternalInput").ap()
    outT = nc.dram_tensor("outT", [E, S], F32, kind="ExternalOutput").ap()

    mm = nc.tensor.matmul

    with ExitStack() as ctx:
        consts = ctx.enter_context(tc.tile_pool(name="consts", bufs=1))
        xp = ctx.enter_context(tc.tile_pool(name="xp", bufs=1))
        qkvp = ctx.enter_context(tc.tile_pool(name="qkvp", bufs=1))
        psum = ctx.enter_context(tc.tile_pool(name="psum", bufs=3, space="PSUM"))
        psum_o = ctx.enter_context(tc.tile_pool(name="psum_o", bufs=1, space="PSUM"))
        expp = ctx.enter_context(tc.tile_pool(name="expp", bufs=2))
        s1p = ctx.enter_context(tc.tile_pool(name="s1p", bufs=1))
        smallp = ctx.enter_context(tc.tile_pool(name="smallp", bufs=2))
        otnp = ctx.enter_context(tc.tile_pool(name="otnp", bufs=6))
        outfp = ctx.enter_context(tc.tile_pool(name="outfp", bufs=3))

        # ---- input loads: wv + x first (V runs first), spread across queues
        wq_sb = consts.tile([P, EC, DHG], BF16)
        wk_sb = consts.tile([P, EC, DHG], BF16)
        wv_sb = consts.tile([P, EC, DHG], BF16)
        wo_sb = consts.tile([P, NHP, E], BF16)
        ones = consts.tile([P, 64], BF16)
        nc.vector.memset(ones, 1.0)
        nc.scalar.dma_start(wv_sb, wvT.rearrange("(c p) d -> p c d", p=P))
        x_sb = xp.tile([P, EC, S], BF16)
        dma_eng = [nc.gpsimd, nc.sync, nc.scalar]
        for ec in range(EC):
            dma_eng[ec % 3].dma_start(x_sb[:, ec, :], xT[ec * P:(ec + 1) * P, :])
        nc.sync.dma_start(wq_sb, wqT.rearrange("(c p) d -> p c d", p=P))
        nc.gpsimd.dma_start(wk_sb, wkT.rearrange("(c p) d -> p c d", p=P))
        nc.gpsimd.dma_start(wo_sb, woT.rearrange("(h p) e -> p h e", p=P))

        qt_sb = qkvp.tile([P, NHP, S], BF16)   # [dh-pair, hp, s]
        kt_sb = qkvp.tile([P, NHP, S], BF16)
        v_sb = qkvp.tile([P, KC, DHG], BF16)   # [s%128, s-chunk, dh]

        # deferred-work queue: (approx_pe_ns, closure) drained into kc slots
        pend = []

        def drain(budget):
            spent = 0
            while pend and spent < budget:
                cost, fn = pend.pop(0)
                fn()
                spent += cost

        # ---- phase 1 emitters (plain 128x128 array mode) ----
        def emit_v(sc2):
            ps = psum.tile([P, 2, QCW], F32, tag="ps", name="psV")
            for i in range(2):
                sc = 2 * sc2 + i
                for ec in range(EC):
                    mm(ps[:, i, 0:DHG],
                       lhsT=x_sb[:, ec, sc * P:(sc + 1) * P],
                       rhs=wv_sb[:, ec, :],
                       start=(ec == 0), stop=(ec == EC - 1),
                       skip_group_check=True)
            nc.vector.tensor_copy(v_sb[:, 2 * sc2:2 * sc2 + 2, :], ps[:, :, 0:DHG])

        def emit_qk(hp, w_sb, dst, sc2):
            ps = psum.tile([P, 2, QCW], F32, tag="ps", name="psQK")
            for i in range(2):
                ssl = slice((2 * sc2 + i) * QCW, (2 * sc2 + i + 1) * QCW)
                for ec in range(EC):
                    mm(ps[:, i, :],
                       lhsT=w_sb[:, ec, hp * P:(hp + 1) * P],
                       rhs=x_sb[:, ec, ssl],
                       start=(ec == 0), stop=(ec == EC - 1),
                       skip_group_check=True)
            dsl = dst[:, hp, 2 * sc2 * QCW:(2 * sc2 + 2) * QCW]
            nc.vector.tensor_copy(dsl.rearrange("p (c q) -> p c q", c=2), ps)

        # serial prologue: V + Q/K for head-pair 0
        for sc2 in range(KC // 2):
            emit_v(sc2)
        for w_sb, dst in ((wq_sb, qt_sb), (wk_sb, kt_sb)):
            for sc2 in range(NQC // 2):
                emit_qk(0, w_sb, dst, sc2)
        # Q/K for head-pair 1: deferred into head-pair 0's attention slots
        for w_sb, dst in ((wq_sb, qt_sb), (wk_sb, kt_sb)):
            for sc2 in range(NQC // 2):
                pend.append((3400, lambda w=w_sb, d=dst, s=sc2: emit_qk(1, w, d, s)))

        # ---- phase 2: attention (64x64 array mode), deferred tails ----
        otn_tiles = {}

        def make_tail(hp, qc, exp_t, psO):
            def evac():
                ot = smallp.tile([P, QCW], F32, tag="ot")
                nc.vector.tensor_copy(ot, psO[:, 0, :])
                otf = smallp.tile([P, QCW], F32, tag="otf")
                nc.vector.tensor_add(otf, psO[:, 1, :], ot)
                otn_tiles[(hp, qc)] = (otf, None)

            def ones_blk(j0):
                s1 = otn_tiles[(hp, qc, "s1")]
                if j0 == 0:
                    otn_tiles[(hp, qc, "psS")] = psum.tile(
                        [P, 2, QCW], F32, tag="ps", name="psS")
                psS = otn_tiles[(hp, qc, "psS")]
                for j in range(j0, j0 + 4):
                    for h in range(2):
                        for g in range(2):
                            rg = slice(g * 64, (g + 1) * 64)
                            mm(psS[h * 64:(h + 1) * 64, g, :],
                               lhsT=ones[rg, :],
                               rhs=s1[rg, h, j, :],
                               start=(j == 0), stop=(j == 7),
                               tile_position=(g * 64, h * 64),
                               skip_group_check=True)

            def norm():
                psS = otn_tiles[(hp, qc, "psS")]
                st = smallp.tile([P, QCW], F32, tag="st")
                nc.vector.tensor_copy(st, psS[:, 0, :])
                ssum = smallp.tile([P, QCW], F32, tag="ssum")
                nc.vector.tensor_add(ssum, psS[:, 1, :], st)
                rec = smallp.tile([P, QCW], F32, tag="rec")
                rscr = smallp.tile([P, QCW], F32, tag="rscr")
                nc.vector.reciprocal_approx_accurate(rec, ssum, rscr)
                otf, _ = otn_tiles[(hp, qc)]
                otn = otnp.tile([P, QCW], BF16)
                nc.vector.tensor_mul(otn, otf, rec)
                otn_tiles[(hp, qc)] = otn

            return [(200, evac),
                    (900, lambda: ones_blk(0)), (900, lambda: ones_blk(4)),
                    (350, norm)]

        def make_outproj(qc):
            qsl = slice(qc * QCW, (qc + 1) * QCW)

            def blk(m2):
                psP = psum.tile([P, 2, QCW], F32, tag="ps", name="psP")
                for i in range(2):
                    m = 2 * m2 + i
                    for hp2 in range(NHP):
                        mm(psP[:, i, :],
                           lhsT=wo_sb[:, hp2, m * P:(m + 1) * P],
                           rhs=otn_tiles[(hp2, qc)],
                           start=(hp2 == 0), stop=(hp2 == NHP - 1),
                           skip_group_check=True)
                outf = outfp.tile([P, 2, QCW], F32)
                nc.vector.tensor_copy(outf, psP)
                (nc.sync if m2 % 2 == 0 else nc.gpsimd).dma_start(
                    outT[2 * m2 * P:(2 * m2 + 2) * P, qsl].rearrange(
                        "(c p) q -> p c q", p=P),
                    outf)

            return [(1900, lambda m=m2: blk(m)) for m2 in range(EC // 2)]

        for hp in range(NHP):
            for qc in range(NQC):
                qsl = slice(qc * QCW, (qc + 1) * QCW)
                exp_t = expp.tile([P, 2, KC, QCW], BF16)
                psO = psum_o.tile([P, 2, QCW], F32)
                for kcs in range(KC + 1):
                    if kcs < KC:
                        c = kcs
                        psL = psum.tile([P, 2, QCW], F32, tag="ps", name="psL")
                        for h in range(2):      # head of the pair
                            hg = slice(h * 64, (h + 1) * 64)
                            for p2 in range(2):  # key sub-chunk of 64
                                mm(psL[p2 * 64:(p2 + 1) * 64, h, :],
                                   lhsT=kt_sb[hg, hp, c * P + p2 * 64:c * P + (p2 + 1) * 64],
                                   rhs=qt_sb[hg, hp, qsl],
                                   start=True, stop=True,
                                   tile_position=(h * 64, p2 * 64),
                                   skip_group_check=True)
                        nc.scalar.activation(exp_t[:, :, c, :], psL, EXP, scale=SCALE)
                    drain(700)
                    if kcs >= 1:
                        c = kcs - 1
                        for h in range(2):
                            col = hp * P + h * 64
                            for g in range(2):  # key sub-chunk = row group
                                rg = slice(g * 64, (g + 1) * 64)
                                mm(psO[h * 64:(h + 1) * 64, g, :],
                                   lhsT=v_sb[rg, c, col:col + 64],
                                   rhs=exp_t[rg, h, c, :],
                                   start=(c == 0), stop=(c == KC - 1),
                                   tile_position=(g * 64, h * 64),
                                   skip_group_check=True)
                    if kcs >= 9:
                        j = kcs - 9
                        if j == 0:
                            otn_tiles[(hp, qc, "s1")] = s1p.tile(
                                [P, 2, 8, QCW], BF16, name="s1", tag="s1")
                        s1 = otn_tiles[(hp, qc, "s1")]
                        nc.vector.tensor_add(
                            s1[:, :, j, :], exp_t[:, :, j, :],
                            exp_t[:, :, j + 8, :])
                pend.extend(make_tail(hp, qc, exp_t, psO))
                if hp == 1:
                    pend.extend(make_outproj(qc))
        drain(10**9)


def _build():
    nc = bacc.Bacc("TRN2", debug=False, target_bir_lowering=False)
    with tile.TileContext(nc) as tc:
        _emit(tc)
    nc.compile()
    return nc


def _get_nc():
    global _NC
    if _NC is None:
        _NC = _build()
    return _NC


def make_in_maps(x, Wq, Wk, Wv, Wo):
    bf = ml_dtypes.bfloat16
    x = np.asarray(x, np.float32)
    xTb = [np.ascontiguousarray(x[b].T).astype(bf) for b in range(B)]
    WqT = np.ascontiguousarray(np.asarray(Wq, np.float32).T).astype(bf)
    WkT = np.ascontiguousarray(np.asarray(Wk, np.float32).T).astype(bf)
    WvT = np.ascontiguousarray(np.asarray(Wv, np.float32).T).astype(bf)
    WoT = np.ascontiguousarray(np.asarray(Wo, np.float32).T).astype(bf)

    in_maps = []
    for c in range(NCORES):
        b, hg = divmod(c, GROUPS)
        sl = slice(hg * DHG, (hg + 1) * DHG)
        in_maps.append({
            "xT": xTb[b],
            "wqT": np.ascontiguousarray(WqT[:, sl]),
            "wkT": np.ascontiguousarray(WkT[:, sl]),
            "wvT": np.ascontiguousarray(WvT[:, sl]),
            "woT": np.ascontiguousarray(WoT[sl, :]),
        })
    return in_maps


def run(in_maps, **kwargs):
    nc = _get_nc()
    return bass_utils.run_bass_kernel_spmd(
        nc, in_maps, core_ids=list(range(NCORES)), **kwargs)


def assemble(outs, bo):
    bo = np.asarray(bo, np.float32)
    out = np.empty((B, S, E), np.float32)
    for b in range(B):
        acc = outs[b * GROUPS]["outT"].copy()
        for hg in range(1, GROUPS):
            acc += outs[b * GROUPS + hg]["outT"]
        out[b] = acc.T + bo
    return out


def kernel(x, Wq, Wk, Wv, Wo, bo):
    in_maps = make_in_maps(x, Wq, Wk, Wv, Wo)
    res = run(in_maps)
    return assemble(res.results, bo)
